# revision 19
# baseline (speedup 1.0000x reference)
"""Multi-head self-attention (B=2, T=2048, D=1024, H=16, causal) on 8 TRN2
NeuronCores.

Sharding: data parallel over batch (2) x tensor parallel over heads (4 groups
of 4 heads) = 8 cores. Each core computes qkv projection for its 4 heads, the
causal attention, and a partial out-projection over its heads' channels; the
host sums the 4 partials per batch and adds b_out.

Device layouts (per core):
  xT      [D=1024, T=2048]   x[b] transposed (host)
  wqkT    [D, 512]           q/k projection weights, chan order
                             [q(h0)|q(h1)] [k(h0)|k(h1)] [q(h2)|q(h3)] [k(h2)|k(h3)]
                             (64 rows each; q rows pre-scaled by 1/sqrt(HD))
  wvT     [D, 256]           v weights [v(h0)|v(h1)|v(h2)|v(h3)]
  woutT   [256, D]           W_out columns for this core's heads, transposed
  mask    [128, 128]         mask[r, c] = 1.0 if r <= c else 0 (causal, within-tile)
Output:
  outT    [D, T]             partial (pre-bias) out-projection, transposed

Attention per head: scores^T tiles [tk=128, tq=512] = kT.T @ qT (K=HD=64),
exp on ScalarE (scores are bounded, no max-subtraction needed), causal mask by
column-trimming + one 128x128 triangular mask multiply on the staircase block,
then out'^T [65, tq] = v_aug.T @ exp^T accumulated over tk tiles where v_aug
is v with a ones column appended - row 64 of the result is sum(exp), used to
normalize. All matmuls run as float32r (full PE rate at free dim >= 256).
"""

import numpy as np

import concourse.bass as bass
import concourse.tile as tile
from concourse import mybir
from concourse.bass_utils import run_bass_kernel_spmd

B, T, D, H = 2, 2048, 1024, 16
HD = D // H  # 64
NCORES = 8
HPC = 4  # heads per core
F32 = mybir.dt.float32
F32R = mybir.dt.float32r
EXP = mybir.ActivationFunctionType.Exp

_NTQ = T // 512  # 4 tq stripes of 512
_NTK = T // 128  # 16 tk tiles of 128
_NKD = D // 128  # 8 contraction tiles over D


def _apply_drain_patch():
    """This walrus build rejects >1 sync-wait command on a CTRL/Drain, so
    split the Tile tail-drain's waits across one drain instruction per
    pending proc."""
    import bass_rust

    if getattr(tile.TileContext, "_drain_patch_applied", False):
        return

    def _split_drain_and_barrier(self, tick_clock, wait_clock):
        nc = self.nc
        gc = tick_clock.global_clock
        NP = 27
        ticks = [gc[p] for p in range(NP)]
        for p in range(NP):
            if ticks[p] <= 0:
                continue
            partial = bass_rust.VectorClock(
                [ticks[q] if q == p else 0 for q in range(NP)]
            )
            d = nc.sync.drain()
            wait_clock.add_sem_waits(d.ins, bass_rust.ScopedClock({None: partial}))
        nc.all_engine_barrier()
        assert self.sems is not None
        popped = nc._tile_sem_poison_stack.pop()
        assert popped is self._sem_poison
        nc.clear_and_free_semaphores(list(self.sems.allocated().values()))
        nc.all_engine_barrier()

    tile.TileContext._drain_and_barrier = _split_drain_and_barrier
    tile.TileContext._drain_patch_applied = True


def _split_waits(nc):
    """This walrus build allows only one sync-wait command per instruction;
    move extra waits onto same-engine NOPs inserted right before."""
    import bass_rust

    f = nc.m.functions[0]
    ctr = 0
    for blk in f.blocks:
        insts = list(blk.instructions)
        new = []
        changed = False
        for inst in insts:
            si = getattr(inst, "sync_info", None)
            if si is not None and len(si.on_wait) > 1:
                waits = list(si.on_wait)
                for w in waits[:-1]:
                    nop = mybir.InstNoOp(name=f"wsplit-{ctr}", ins=[], outs=[])
                    ctr += 1
                    nop.engine = inst.engine
                    nop.sync_info = bass_rust.SyncInfo(on_wait=[w], on_update=[])
                    nc.register_instruction(nop, overwrite=True)
                    new.append(nop)
                inst.sync_info = bass_rust.SyncInfo(
                    on_wait=[waits[-1]], on_update=list(si.on_update))
                changed = True
            new.append(inst)
        if changed:
            blk.instructions = new


def build_nc():
    """Build the per-core Bass program (identical on all 8 cores)."""
    _apply_drain_patch()
    nc = bass.Bass("TRN2", target_bir_lowering=False, debug=False,
                   num_devices=NCORES)

    xT = nc.dram_tensor("xT", [D, T], F32R, kind="ExternalInput").ap()
    wqkT = nc.dram_tensor("wqkT", [D, 8 * HD], F32R, kind="ExternalInput").ap()
    bqk = nc.dram_tensor("bqk", [128, 4], F32, kind="ExternalInput").ap()
    wvT = nc.dram_tensor("wvT", [D, 4 * HD], F32R, kind="ExternalInput").ap()
    bv = nc.dram_tensor("bv", [1, 4 * HD], F32, kind="ExternalInput").ap()
    woutT = nc.dram_tensor("woutT", [4 * HD, D], F32R, kind="ExternalInput").ap()
    mask = nc.dram_tensor("mask", [128, 128], F32, kind="ExternalInput").ap()
    one = nc.dram_tensor("one", [1, 1], F32R, kind="ExternalInput").ap()
    outT = nc.dram_tensor("outT", [D, T], F32, kind="ExternalOutput").ap()

    with tile.TileContext(nc) as tc:
        _emit(nc, tc, xT, wqkT, bqk, wvT, bv, woutT, mask, one, outT)
    _split_waits(nc)
    return nc


def _emit(nc, tc, xT, wqkT, bqk, wvT, bv, woutT, mask, one, outT):
    import contextlib

    with contextlib.ExitStack() as ctx:
        const = ctx.enter_context(tc.tile_pool(name="const", bufs=1))
        persist = ctx.enter_context(tc.tile_pool(name="persist", bufs=1))

        wqk_sb = const.tile([128, _NKD, 8 * HD], F32R)
        nc.sync.dma_start(out=wqk_sb, in_=wqkT.rearrange("(k p) c -> p k c", p=128))
        wv_sb = const.tile([128, _NKD, 4 * HD], F32R)
        nc.sync.dma_start(out=wv_sb, in_=wvT.rearrange("(k p) c -> p k c", p=128))
        wo_sb = const.tile([128, 2, D], F32R)
        nc.sync.dma_start(out=wo_sb, in_=woutT.rearrange("(k p) c -> p k c", p=128))
        mask_sb = const.tile([128, 128], F32)
        nc.sync.dma_start(out=mask_sb, in_=mask)
        bqk_sb = const.tile([128, 4], F32)
        nc.sync.dma_start(out=bqk_sb, in_=bqk)
        bv_sb = const.tile([128, 4 * HD], F32)
        nc.gpsimd.dma_start(
            out=bv_sb,
            in_=bass.AP(tensor=bv.tensor, offset=bv.offset,
                        ap=[[0, 128], [1, 4 * HD]]),
        )

        # qkT[:, m, :]: m=0 -> q(h0)|q(h1), 1 -> k(h0)|k(h1), 2 -> q(h2)|q(h3),
        # 3 -> k(h2)|k(h3); partition p<64 is head h0/h2, p>=64 is h1/h3.
        qkT = persist.tile([128, 4, T], F32R)
        # v with a trailing ones column: [tq-part, tk-tile, head, HD+1]
        # (sum(exp) lands on psum partition 64)
        vaug = persist.tile([128, _NTK, HPC, HD + 1], F32R)
        nc.gpsimd.dma_start(
            out=vaug[:, :, :, HD:HD + 1],
            in_=bass.AP(tensor=one.tensor, offset=one.offset,
                        ap=[[0, 128], [0, _NTK * HPC], [0, 1]]),
        )
        # attention output^T, stacked [h0|h1] / [h2|h3] on partitions
        aT = persist.tile([128, 2, T], F32R)

        # ---- Phase A: qkv projections ----
        with tc.tile_pool(name="xp", bufs=1) as xp, \
             tc.tile_pool(name="psA", bufs=3, space="PSUM") as psA, \
             tc.tile_pool(name="psV", bufs=2, space="PSUM") as psV:
            xT_sb = xp.tile([128, _NKD, T], F32R)
            nc.sync.dma_start(out=xT_sb, in_=xT.rearrange("(k p) t -> p k t", p=128))
            for n in range(_NTQ):
                for m in range(4):
                    ps = psA.tile([128, 512], F32, tag="qk")
                    for k in range(_NKD):
                        nc.tensor.matmul(
                            ps,
                            lhsT=wqk_sb[:, k, m * 128:(m + 1) * 128],
                            rhs=xT_sb[:, k, n * 512:(n + 1) * 512],
                            start=(k == 0), stop=(k == _NKD - 1),
                        )
                    nc.vector.tensor_scalar_add(
                        out=qkT[:, m, n * 512:(n + 1) * 512],
                        in0=ps, scalar1=bqk_sb[:, m:m + 1],
                    )
            for t in range(_NTK):
                psv = psV.tile([128, 4 * HD], F32, tag="v")
                for k in range(_NKD):
                    nc.tensor.matmul(
                        psv,
                        lhsT=xT_sb[:, k, t * 128:(t + 1) * 128],
                        rhs=wv_sb[:, k, :],
                        start=(k == 0), stop=(k == _NKD - 1),
                    )
                nc.vector.tensor_add(
                    out=vaug[:, t, :, 0:HD],
                    in0=psv.rearrange("p (h d) -> p h d", h=HPC),
                    in1=bv_sb.rearrange("p (h d) -> p h d", h=HPC),
                )

        # ---- Phase B+C: attention per (stripe j, head h), then out_proj(j) ----
        with tc.tile_pool(name="psS", bufs=2, space="PSUM") as psS, \
             tc.tile_pool(name="psAV", bufs=2, space="PSUM") as psAV, \
             tc.tile_pool(name="psO", bufs=2, space="PSUM") as psO, \
             tc.tile_pool(name="expp", bufs=4) as expp, \
             tc.tile_pool(name="small", bufs=3) as small, \
             tc.tile_pool(name="dscr", bufs=3, space="DRAM") as dscr, \
             tc.tile_pool(name="outp", bufs=3) as outp:
            for j in range(_NTQ):
                for h in range(HPC):
                    pair, sub = h // 2, h % 2
                    qT_h = qkT[sub * 64:(sub + 1) * 64, 2 * pair, :]
                    kT_h = qkT[sub * 64:(sub + 1) * 64, 2 * pair + 1, :]
                    ps_av = psAV.tile([HD + 1, 512], F32, tag="av")
                    ntk = 4 * j + 4
                    for i in range(ntk):
                        a = i - 4 * j  # >= 0 on the causal staircase
                        ps_s = psS.tile([128, 512], F32, tag="s")
                        nc.tensor.matmul(
                            ps_s,
                            lhsT=kT_h[:, i * 128:(i + 1) * 128],
                            rhs=qT_h[:, j * 512:(j + 1) * 512],
                            start=True, stop=True,
                        )
                        expT = expp.tile([128, 512], F32R, tag="e")
                        col0 = max(a, 0) * 128
                        nc.scalar.activation(expT[:, col0:512], ps_s[:, col0:512], EXP)
                        if a >= 0:
                            nc.vector.tensor_mul(
                                expT[:, col0:col0 + 128],
                                expT[:, col0:col0 + 128], mask_sb)
                        nc.tensor.matmul(
                            ps_av[:, col0:512],
                            lhsT=vaug[:, i, h, :],
                            rhs=expT[:, col0:512],
                            start=(i == 0), stop=(i == ntk - 1),
                        )
                    # normalize: rows 0..63 are out'^T, row 64 is sum(exp)
                    rec = small.tile([HD + 1, 512], F32, tag="rec")
                    nc.vector.reciprocal(rec[HD:HD + 1, :], ps_av[HD:HD + 1, :])
                    # partition-broadcast via DRAM bounce (step-0 partition
                    # APs are only legal with a DRAM source)
                    dtmp = dscr.tile([1, 512], F32, tag="dt")
                    nc.sync.dma_start(out=dtmp, in_=rec[HD:HD + 1, :])
                    rb = small.tile([HD, 512], F32, tag="rb")
                    nc.gpsimd.dma_start(
                        out=rb,
                        in_=bass.AP(tensor=dtmp.tensor, offset=dtmp.offset,
                                    ap=[[0, HD]] + [list(p) for p in dtmp.ap[1:]]),
                    )
                    tmp = small.tile([HD, 512], F32R, tag="tmp")
                    nc.vector.tensor_mul(tmp, ps_av[0:HD, :], rb)
                    nc.sync.dma_start(
                        out=aT[sub * 64:(sub + 1) * 64, pair,
                               j * 512:(j + 1) * 512],
                        in_=tmp)
                for m in range(D // 128):
                    po = psO.tile([128, 512], F32, tag="o")
                    for kk in range(2):
                        nc.tensor.matmul(
                            po,
                            lhsT=wo_sb[:, kk, m * 128:(m + 1) * 128],
                            rhs=aT[:, kk, j * 512:(j + 1) * 512],
                            start=(kk == 0), stop=(kk == 1),
                        )
                    ot = outp.tile([128, 512], F32, tag="ot")
                    nc.vector.tensor_copy(ot, po)
                    nc.sync.dma_start(
                        out=outT[m * 128:(m + 1) * 128, j * 512:(j + 1) * 512],
                        in_=ot)


def shard_inputs(x, W_qkv, b_qkv, W_out):
    """Host-side packing: one input dict per core."""
    x = np.asarray(x, np.float32)
    Wr = np.asarray(W_qkv, np.float32).reshape(H, 3, HD, D)
    br = np.asarray(b_qkv, np.float32).reshape(H, 3, HD)
    W_out = np.asarray(W_out, np.float32)
    scale = 1.0 / np.sqrt(HD)

    mask128 = np.triu(np.ones((128, 128), np.float32))
    in_maps = []
    for c in range(NCORES):
        b, g = divmod(c, 4)
        hh = [4 * g + i for i in range(HPC)]
        # chan-tile order: q(h0)|q(h1), k(h0)|k(h1), q(h2)|q(h3), k(h2)|k(h3)
        qk_rows, qk_bias = [], []
        for p in range(2):
            h0, h1 = hh[2 * p], hh[2 * p + 1]
            qk_rows += [Wr[h0, 0] * scale, Wr[h1, 0] * scale, Wr[h0, 1], Wr[h1, 1]]
            qk_bias += [br[h0, 0] * scale, br[h1, 0] * scale, br[h0, 1], br[h1, 1]]
        wqk = np.concatenate(qk_rows, 0)          # [512, D]
        bqk = np.concatenate(qk_bias, 0)          # [512]
        wv = np.concatenate([Wr[h, 2] for h in hh], 0)   # [256, D]
        bvv = np.concatenate([br[h, 2] for h in hh], 0)  # [256]
        cols = np.concatenate([np.arange(h * HD, (h + 1) * HD) for h in hh])
        in_maps.append({
            "xT": np.ascontiguousarray(x[b].T),
            "wqkT": np.ascontiguousarray(wqk.T),
            "bqk": np.ascontiguousarray(bqk.reshape(4, 128).T),
            "wvT": np.ascontiguousarray(wv.T),
            "bv": np.ascontiguousarray(bvv.reshape(1, 4 * HD)),
            "woutT": np.ascontiguousarray(W_out[:, cols].T),
            "mask": mask128,
            "one": np.ones((1, 1), np.float32),
        })
    return in_maps


_NC = None


def kernel(x, mask, W_qkv, b_qkv, W_out, b_out, **run_kwargs):
    global _NC
    if _NC is None:
        _NC = build_nc()
    in_maps = shard_inputs(x, W_qkv, b_qkv, W_out)
    res = run_bass_kernel_spmd(_NC, in_maps, core_ids=list(range(NCORES)),
                               **run_kwargs)
    b_out = np.asarray(b_out, np.float64)
    outs = []
    for b in range(B):
        acc = np.zeros((D, T), np.float64)
        for g in range(4):
            acc += res.results[4 * b + g]["outT"]
        outs.append(acc.T + b_out[None, :])
    out = np.stack(outs).astype(np.float32)
    if run_kwargs:
        kernel.last_results = res
    return out
